# revision 9
# baseline (speedup 1.0000x reference)
"""Depthwise 3D conv (3x3x3, SAME, C=64) on 8 Trainium2 NeuronCores.

Strategy
--------
Data-parallel over (batch, d-half): core k handles b = k//2 and output
frames d in [8*(k%2), 8*(k%2)+8). The d-halo (1 frame each side) is
zero-padded on host so every core runs the identical 10-input-frame
program (SAME padding at batch edges falls out of the zero frames).

Per channel c the 27 taps factor into 9 TensorE matmuls, one per
(kd, kw): contraction over h_in with a per-channel banded matrix
B[h_in, h_out] that carries the 3 kh taps on its diagonals, PSUM-
accumulating all 9 into one [h=112, (d=4, w=112)] tile. The kd/kw
shifts are plain access-pattern offsets on the moving operand. Band
matrices are built on host (they're just w values on 3 diagonals) and
DMA'd in bf16; x is host-transposed to [h, c, d, w] bf16 (h outermost
so chunked DMAs get multi-KB contiguous runs per partition). Output
fp32.
"""

import json
import sys
import types

if "/opt/trn_rl_repo" not in sys.path:
    sys.path.insert(0, "/opt/trn_rl_repo")

import ml_dtypes
import numpy as np

KD = KH = KW = 3
C = 64
B_FULL, D_FULL, H, W = 4, 16, 112, 112
N_CORES = 8
D_OUT = 8  # output frames per core
D_IN = D_OUT + 2  # with zero-padded halo
DBLK = 4  # output frames per psum accumulation group
N_DBLK = D_OUT // DBLK
CG = 8  # channels per input DMA chunk
OG = 4  # channels per output DMA chunk
BF16 = ml_dtypes.bfloat16

_TAPS = [(0, 1), (0, 0), (0, 2), (1, 0), (1, 1), (1, 2), (2, 0), (2, 1), (2, 2)]


def _legalize_bir(raw: bytes) -> bytes:
    """walrus in this image caps sem waits at 1 per instruction; hoist extra
    waits onto preceding same-engine NoOps (sequencers run them in order)."""
    d = json.loads(raw)
    for fn in d["functions"]:
        for blk in fn["blocks"]:
            out = []
            for inst in blk["instructions"]:
                si = inst.get("sync_info")
                waits = (si or {}).get("on_wait") or []
                if len(waits) > 1:
                    for j, wt in enumerate(waits[:-1]):
                        out.append(
                            {
                                "debug": inst.get("debug", 0),
                                "engine": inst["engine"],
                                "ins": [],
                                "outs": [],
                                "name": f"{inst['name']}-w{j}",
                                "opcode": "NoOp",
                                "sync_info": {"on_wait": [wt], "on_update": []},
                            }
                        )
                    si["on_wait"] = [waits[-1]]
                out.append(inst)
            blk["instructions"] = out
    return json.dumps(d).encode()


def _build_nc():
    import concourse.bass as bass
    import concourse.mybir as mybir
    import concourse.tile as tile

    nc = bass.Bass()
    x_d = nc.declare_dram_parameter(
        "x", [H, C, D_IN, W], mybir.dt.bfloat16, isOutput=False
    )
    b_d = nc.declare_dram_parameter(
        "bmat", [H, C, len(_TAPS), H], mybir.dt.bfloat16, isOutput=False
    )
    y_d = nc.declare_dram_parameter("y", [H, C, D_OUT, W], mybir.dt.float32, isOutput=True)

    with tile.TileContext(nc) as tc:
        with (
            tc.tile_pool(name="xin", bufs=3) as xin_pool,
            tc.tile_pool(name="bmat", bufs=3) as b_pool,
            tc.tile_pool(name="psum", bufs=6, space="PSUM") as psum_pool,
            tc.tile_pool(name="osb", bufs=3) as osb_pool,
        ):
            for cg in range(C // CG):
                xt = xin_pool.tile([H, CG, D_IN, W], mybir.dt.bfloat16)
                bt = b_pool.tile([H, CG, len(_TAPS), H], mybir.dt.bfloat16)
                nc.sync.dma_start(out=xt[:], in_=x_d[:, cg * CG : (cg + 1) * CG])
                nc.sync.dma_start(out=bt[:], in_=b_d[:, cg * CG : (cg + 1) * CG])
                for oi in range(CG // OG):
                    osb = osb_pool.tile([H, OG, D_OUT, W], mybir.dt.float32)
                    for ci in range(OG):
                        cc = oi * OG + ci
                        for db in range(N_DBLK):
                            ps = psum_pool.tile([H, DBLK, W], mybir.dt.float32)
                            for i, (kd, kw) in enumerate(_TAPS):
                                d_lo = db * DBLK + kd
                                if kw == 1:
                                    wi, wj, wo, wp = 0, W, 0, W
                                elif kw == 0:
                                    wi, wj, wo, wp = 0, W - 1, 1, W
                                else:
                                    wi, wj, wo, wp = 1, W, 0, W - 1
                                t = kd * 3 + kw
                                nc.tensor.matmul(
                                    ps[:, :, wo:wp],
                                    bt[:, cc, t, :],
                                    xt[:, cc, d_lo : d_lo + DBLK, wi:wj],
                                    start=(i == 0),
                                    stop=(i == len(_TAPS) - 1),
                                    skip_group_check=(i != 0),
                                )
                            nc.vector.tensor_copy(
                                osb[:, ci, db * DBLK : (db + 1) * DBLK, :],
                                ps[:],
                            )
                    c0 = cg * CG + oi * OG
                    nc.scalar.dma_start(out=y_d[:, c0 : c0 + OG], in_=osb[:])

    orig_to_json = nc.to_json_bytes
    nc.to_json_bytes = types.MethodType(
        lambda self: _legalize_bir(orig_to_json()), nc
    )
    return nc


def _host_prep(x: np.ndarray, w: np.ndarray):
    """Build per-core [h, c, d, w] bf16 inputs and the band matrices."""
    # x: (4, 16, 112, 112, 64) f32; w: (3, 3, 3, 1, 64) f32
    xt = np.ascontiguousarray(np.transpose(x, (0, 2, 4, 1, 3)))  # (b, h, c, d, w)

    wt = w[:, :, :, 0, :].astype(np.float32)  # (kd, kh, kw, c)
    bmat = np.zeros((H, C, len(_TAPS), H), np.float32)
    ho = np.arange(H)
    for kd in range(KD):
        for kw in range(KW):
            t = kd * 3 + kw
            for kh in range(KH):
                sel = ho[(ho + kh - 1 >= 0) & (ho + kh - 1 < H)]
                bmat[sel + kh - 1, :, t, sel] = wt[kd, kh, kw, :]
    bmat = bmat.astype(BF16)

    in_maps = []
    for k in range(N_CORES):
        b = k // 2
        d0 = (k % 2) * D_OUT
        lo, hi = d0 - 1, d0 + D_OUT + 1
        clo, chi = max(lo, 0), min(hi, D_FULL)
        xc = np.zeros((H, C, D_IN, W), BF16)
        xc[:, :, clo - lo : clo - lo + (chi - clo), :] = xt[b, :, :, clo:chi, :].astype(
            BF16
        )
        in_maps.append({"x": xc, "bmat": bmat})
    return in_maps


def _assemble(results):
    y = np.empty((B_FULL, D_FULL, H, W, C), np.float32)
    for k in range(N_CORES):
        b = k // 2
        d0 = (k % 2) * D_OUT
        # y core layout: (h, c, d, w) -> (d, h, w, c)
        y[b, d0 : d0 + D_OUT] = np.transpose(results[k]["y"], (2, 0, 3, 1))
    return y


def _run(x: np.ndarray, w: np.ndarray, trace: bool = False):
    from concourse.bass_utils import run_bass_kernel_spmd

    in_maps = _host_prep(np.asarray(x), np.asarray(w))
    nc = _build_nc()
    res = run_bass_kernel_spmd(nc, in_maps, list(range(N_CORES)), trace=trace)
    return _assemble(res.results), res.exec_time_ns


def kernel(x: np.ndarray, w: np.ndarray) -> np.ndarray:
    y, _ = _run(x, w, trace=False)
    return y


# revision 11
# speedup vs baseline: 1.0602x; 1.0602x over previous
"""Depthwise 3D conv (3x3x3, SAME, C=64) on 8 Trainium2 NeuronCores.

Strategy
--------
Data-parallel over (batch, d-half): core k handles b = k//2 and output
frames d in [8*(k%2), 8*(k%2)+8). The d-halo (1 frame each side) is
zero-padded on host so every core runs the identical 10-input-frame
program (SAME padding at batch edges falls out of the zero frames).

Per channel c the 27 taps factor into 9 TensorE matmuls, one per
(kd, kw): contraction over h_in with a per-channel banded matrix
B[h_in, h_out] that carries the 3 kh taps on its diagonals, PSUM-
accumulating all 9 into one [h=112, (d=4, w=112)] tile. The kd/kw
shifts are plain access-pattern offsets on the moving operand. Band
matrices are built on host (they're just w values on 3 diagonals) and
DMA'd in bf16; x is host-transposed to [h, c, d, w] bf16 (h outermost
so chunked DMAs get multi-KB contiguous runs per partition). Output
fp32.
"""

import json
import sys
import types

if "/opt/trn_rl_repo" not in sys.path:
    sys.path.insert(0, "/opt/trn_rl_repo")

import ml_dtypes
import numpy as np

KD = KH = KW = 3
C = 64
B_FULL, D_FULL, H, W = 4, 16, 112, 112
N_CORES = 8
D_OUT = 8  # output frames per core
D_IN = D_OUT + 2  # with zero-padded halo
DBLK = 4  # output frames per psum accumulation group
N_DBLK = D_OUT // DBLK
CG = 8  # channels per input DMA chunk
OG = 4  # channels per output DMA chunk
BF16 = ml_dtypes.bfloat16

_TAPS = [(0, 1), (0, 0), (0, 2), (1, 0), (1, 1), (1, 2), (2, 0), (2, 1), (2, 2)]


def _legalize_bir(raw: bytes) -> bytes:
    """walrus in this image caps sem waits at 1 per instruction; hoist extra
    waits onto preceding same-engine NoOps (sequencers run them in order)."""
    d = json.loads(raw)
    for fn in d["functions"]:
        for blk in fn["blocks"]:
            out = []
            for inst in blk["instructions"]:
                si = inst.get("sync_info")
                waits = (si or {}).get("on_wait") or []
                if len(waits) > 1:
                    for j, wt in enumerate(waits[:-1]):
                        out.append(
                            {
                                "debug": inst.get("debug", 0),
                                "engine": inst["engine"],
                                "ins": [],
                                "outs": [],
                                "name": f"{inst['name']}-w{j}",
                                "opcode": "NoOp",
                                "sync_info": {"on_wait": [wt], "on_update": []},
                            }
                        )
                    si["on_wait"] = [waits[-1]]
                out.append(inst)
            blk["instructions"] = out
    return json.dumps(d).encode()


def _build_nc():
    import concourse.bass as bass
    import concourse.mybir as mybir
    import concourse.tile as tile

    nc = bass.Bass()
    x_d = nc.declare_dram_parameter(
        "x", [H, C, D_IN, W], mybir.dt.bfloat16, isOutput=False
    )
    b_d = nc.declare_dram_parameter(
        "bmat", [H, C, len(_TAPS), H], mybir.dt.bfloat16, isOutput=False
    )
    y_d = nc.declare_dram_parameter("y", [H, C, D_OUT, W], mybir.dt.float32, isOutput=True)

    with tile.TileContext(nc) as tc:
        with (
            tc.tile_pool(name="xin", bufs=3) as xin_pool,
            tc.tile_pool(name="bmat", bufs=3) as b_pool,
            tc.tile_pool(name="psum", bufs=6, space="PSUM") as psum_pool,
            tc.tile_pool(name="osb", bufs=3) as osb_pool,
        ):
            # ragged chunking: small first chunks so the PE starts early
            chunks = []
            c0 = 0
            for sz in [2, 2, 4] + [CG] * C:
                if c0 >= C:
                    break
                sz = min(sz, C - c0)
                chunks.append((c0, sz))
                c0 += sz
            for c0, csz in chunks:
                xt = xin_pool.tile([H, CG, D_IN, W], mybir.dt.bfloat16, tag="xin")
                bt = b_pool.tile([H, CG, len(_TAPS), H], mybir.dt.bfloat16, tag="bt")
                nc.sync.dma_start(
                    out=xt[:, :csz], in_=x_d[:, c0 : c0 + csz]
                )
                nc.sync.dma_start(
                    out=bt[:, :csz], in_=b_d[:, c0 : c0 + csz]
                )
                for oi in range((csz + OG - 1) // OG):
                    og = min(OG, csz - oi * OG)
                    osb = osb_pool.tile([H, OG, D_OUT, W], mybir.dt.float32, tag="osb")
                    for ci in range(og):
                        cc = oi * OG + ci
                        for db in range(N_DBLK):
                            ps = psum_pool.tile([H, DBLK, W], mybir.dt.float32)
                            for i, (kd, kw) in enumerate(_TAPS):
                                d_lo = db * DBLK + kd
                                if kw == 1:
                                    wi, wj, wo, wp = 0, W, 0, W
                                elif kw == 0:
                                    wi, wj, wo, wp = 0, W - 1, 1, W
                                else:
                                    wi, wj, wo, wp = 1, W, 0, W - 1
                                t = kd * 3 + kw
                                nc.tensor.matmul(
                                    ps[:, :, wo:wp],
                                    bt[:, cc, t, :],
                                    xt[:, cc, d_lo : d_lo + DBLK, wi:wj],
                                    start=(i == 0),
                                    stop=(i == len(_TAPS) - 1),
                                    skip_group_check=(i != 0),
                                )
                            nc.vector.tensor_copy(
                                osb[:, ci, db * DBLK : (db + 1) * DBLK, :],
                                ps[:],
                            )
                    yc0 = c0 + oi * OG
                    nc.scalar.dma_start(
                        out=y_d[:, yc0 : yc0 + og], in_=osb[:, :og]
                    )

    orig_to_json = nc.to_json_bytes
    nc.to_json_bytes = types.MethodType(
        lambda self: _legalize_bir(orig_to_json()), nc
    )
    return nc


def _host_prep(x: np.ndarray, w: np.ndarray):
    """Build per-core [h, c, d, w] bf16 inputs and the band matrices."""
    # x: (4, 16, 112, 112, 64) f32; w: (3, 3, 3, 1, 64) f32
    xt = np.ascontiguousarray(np.transpose(x, (0, 2, 4, 1, 3)))  # (b, h, c, d, w)

    wt = w[:, :, :, 0, :].astype(np.float32)  # (kd, kh, kw, c)
    bmat = np.zeros((H, C, len(_TAPS), H), np.float32)
    ho = np.arange(H)
    for kd in range(KD):
        for kw in range(KW):
            t = kd * 3 + kw
            for kh in range(KH):
                sel = ho[(ho + kh - 1 >= 0) & (ho + kh - 1 < H)]
                bmat[sel + kh - 1, :, t, sel] = wt[kd, kh, kw, :]
    bmat = bmat.astype(BF16)

    in_maps = []
    for k in range(N_CORES):
        b = k // 2
        d0 = (k % 2) * D_OUT
        lo, hi = d0 - 1, d0 + D_OUT + 1
        clo, chi = max(lo, 0), min(hi, D_FULL)
        xc = np.zeros((H, C, D_IN, W), BF16)
        xc[:, :, clo - lo : clo - lo + (chi - clo), :] = xt[b, :, :, clo:chi, :].astype(
            BF16
        )
        in_maps.append({"x": xc, "bmat": bmat})
    return in_maps


def _assemble(results):
    y = np.empty((B_FULL, D_FULL, H, W, C), np.float32)
    for k in range(N_CORES):
        b = k // 2
        d0 = (k % 2) * D_OUT
        # y core layout: (h, c, d, w) -> (d, h, w, c)
        y[b, d0 : d0 + D_OUT] = np.transpose(results[k]["y"], (2, 0, 3, 1))
    return y


def _run(x: np.ndarray, w: np.ndarray, trace: bool = False):
    from concourse.bass_utils import run_bass_kernel_spmd

    in_maps = _host_prep(np.asarray(x), np.asarray(w))
    nc = _build_nc()
    res = run_bass_kernel_spmd(nc, in_maps, list(range(N_CORES)), trace=trace)
    return _assemble(res.results), res.exec_time_ns


def kernel(x: np.ndarray, w: np.ndarray) -> np.ndarray:
    y, _ = _run(x, w, trace=False)
    return y


# revision 13
# speedup vs baseline: 1.3688x; 1.2911x over previous
"""Depthwise 3D conv (3x3x3, SAME, C=64) on 8 Trainium2 NeuronCores.

Strategy
--------
Data-parallel over (batch, h-half): core k handles b = k//2 and output
rows h in [56*(k%2), 56*(k%2)+56), all 16 d frames. Both the d-halo and
h-halo are materialized on host (zero-padded at volume edges), so every
core runs an identical program.

TensorE mapping: partitions carry a (d, h) block — input block (8, 16)
= 128 partitions, output block (6, 14) = 84 partitions — and the
stationary operand is a per-(channel, kw) banded matrix B[(d_i, h_i),
(d_o, h_o)] = w[kd = d_i - d_o, kh = h_i - h_o, kw, c], so ONE matmul
applies 9 of the 27 taps; the 3 kw taps are w-shifts on the moving
operand's access pattern, PSUM-accumulated. d = 16 tiles as output
blocks {6, 6, 4}; the ragged 4-block uses a (6, 16) = 96-partition
input block with its own (smaller) band matrices. h = 56 tiles as 4
blocks of 14, carried in the moving free dimension alongside w
(j = 4*112 = 448).

x is host-gathered into the block-partition layout (bf16), band
matrices built on host (bf16), outputs returned fp32.
"""

import json
import sys
import types

if "/opt/trn_rl_repo" not in sys.path:
    sys.path.insert(0, "/opt/trn_rl_repo")

import ml_dtypes
import numpy as np

KD = KH = KW = 3
C = 64
B_FULL, D_FULL, H, W = 4, 16, 112, 112
N_CORES = 8
HH = 56  # output h rows per core
NHB = 4  # h blocks of 14 per core
HBO = 14  # out h rows per block
HBI = 16  # in h rows per block
DBO_M, DBI_M = 6, 8  # main d block: out/in frames
DBO_R, DBI_R = 4, 6  # ragged d block
D0S = [0, 6, 12]  # out-frame starts of the 3 d blocks
PM = DBI_M * HBI  # 128 in-partitions (main)
PMO = DBO_M * HBO  # 84 out-partitions (main)
PR = DBI_R * HBI  # 96 in-partitions (ragged)
PRO = DBO_R * HBO  # 56 out-partitions (ragged)
CG = 8  # channels per input DMA chunk
OG = 4  # channels per output DMA chunk
BF16 = ml_dtypes.bfloat16

_KW_ORDER = [1, 0, 2]  # full-width tap first so PSUM start=True covers all cols


def _legalize_bir(raw: bytes) -> bytes:
    """walrus in this image caps sem waits at 1 per instruction; hoist extra
    waits onto preceding same-engine NoOps (sequencers run them in order)."""
    d = json.loads(raw)
    for fn in d["functions"]:
        for blk in fn["blocks"]:
            out = []
            for inst in blk["instructions"]:
                si = inst.get("sync_info")
                waits = (si or {}).get("on_wait") or []
                if len(waits) > 1:
                    for j, wt in enumerate(waits[:-1]):
                        out.append(
                            {
                                "debug": inst.get("debug", 0),
                                "engine": inst["engine"],
                                "ins": [],
                                "outs": [],
                                "name": f"{inst['name']}-w{j}",
                                "opcode": "NoOp",
                                "sync_info": {"on_wait": [wt], "on_update": []},
                            }
                        )
                    si["on_wait"] = [waits[-1]]
                out.append(inst)
            blk["instructions"] = out
    return json.dumps(d).encode()


def _w_ranges(kw):
    # out[w] += wt[kw] * x[w + kw - 1]
    if kw == 1:
        return 0, W, 0, W
    if kw == 0:
        return 0, W - 1, 1, W
    return 1, W, 0, W - 1


def _build_nc():
    import concourse.bass as bass
    import concourse.mybir as mybir
    import concourse.tile as tile

    nc = bass.Bass()
    xm_d = nc.declare_dram_parameter(
        "xm", [PM, C, 2, NHB, W], mybir.dt.bfloat16, isOutput=False
    )
    xr_d = nc.declare_dram_parameter(
        "xr", [PR, C, NHB, W], mybir.dt.bfloat16, isOutput=False
    )
    bm_d = nc.declare_dram_parameter(
        "bm", [PM, C, KW, PMO], mybir.dt.bfloat16, isOutput=False
    )
    br_d = nc.declare_dram_parameter(
        "br", [PR, C, KW, PRO], mybir.dt.bfloat16, isOutput=False
    )
    ym_d = nc.declare_dram_parameter(
        "ym", [PMO, C, 2, NHB, W], mybir.dt.float32, isOutput=True
    )
    yr_d = nc.declare_dram_parameter(
        "yr", [PRO, C, NHB, W], mybir.dt.float32, isOutput=True
    )

    with tile.TileContext(nc) as tc:
        with (
            tc.tile_pool(name="xin", bufs=3) as xin_pool,
            tc.tile_pool(name="bmat", bufs=3) as b_pool,
            tc.tile_pool(name="psum", bufs=4, space="PSUM") as psum_pool,
            tc.tile_pool(name="osb", bufs=3) as osb_pool,
        ):
            chunks = []
            c0 = 0
            for sz in [2, 2, 4] + [CG] * C:
                if c0 >= C:
                    break
                sz = min(sz, C - c0)
                chunks.append((c0, sz))
                c0 += sz
            for c0, csz in chunks:
                xm = xin_pool.tile([PM, CG, 2, NHB, W], mybir.dt.bfloat16, tag="xm")
                xr = xin_pool.tile([PR, CG, NHB, W], mybir.dt.bfloat16, tag="xr")
                bm = b_pool.tile([PM, CG, KW, PMO], mybir.dt.bfloat16, tag="bm")
                br = b_pool.tile([PR, CG, KW, PRO], mybir.dt.bfloat16, tag="br")
                nc.sync.dma_start(out=xm[:, :csz], in_=xm_d[:, c0 : c0 + csz])
                nc.sync.dma_start(out=xr[:, :csz], in_=xr_d[:, c0 : c0 + csz])
                nc.sync.dma_start(out=bm[:, :csz], in_=bm_d[:, c0 : c0 + csz])
                nc.sync.dma_start(out=br[:, :csz], in_=br_d[:, c0 : c0 + csz])
                for oi in range((csz + OG - 1) // OG):
                    og = min(OG, csz - oi * OG)
                    osm = osb_pool.tile([PMO, OG, 2, NHB, W], mybir.dt.float32, tag="osm")
                    osr = osb_pool.tile([PRO, OG, NHB, W], mybir.dt.float32, tag="osr")
                    for ci in range(og):
                        cc = oi * OG + ci
                        for db in range(2):
                            ps = psum_pool.tile([PMO, NHB, W], mybir.dt.float32, tag="psm")
                            for i, kw in enumerate(_KW_ORDER):
                                wi, wj, wo, wp = _w_ranges(kw)
                                nc.tensor.matmul(
                                    ps[:, :, wo:wp],
                                    bm[:, cc, kw, :],
                                    xm[:, cc, db, :, wi:wj],
                                    start=(i == 0),
                                    stop=(i == KW - 1),
                                    skip_group_check=(i != 0),
                                )
                            if db == 0:
                                nc.vector.tensor_copy(osm[:, ci, db], ps[:])
                            else:
                                nc.scalar.copy(out=osm[:, ci, db], in_=ps[:])
                        psr = psum_pool.tile([PRO, NHB, W], mybir.dt.float32, tag="psr")
                        for i, kw in enumerate(_KW_ORDER):
                            wi, wj, wo, wp = _w_ranges(kw)
                            nc.tensor.matmul(
                                psr[:, :, wo:wp],
                                br[:, cc, kw, :],
                                xr[:, cc, :, wi:wj],
                                start=(i == 0),
                                stop=(i == KW - 1),
                                skip_group_check=(i != 0),
                            )
                        nc.vector.tensor_copy(osr[:, ci], psr[:])
                    yc0 = c0 + oi * OG
                    nc.scalar.dma_start(out=ym_d[:, yc0 : yc0 + og], in_=osm[:, :og])
                    nc.scalar.dma_start(out=yr_d[:, yc0 : yc0 + og], in_=osr[:, :og])

    orig_to_json = nc.to_json_bytes
    nc.to_json_bytes = types.MethodType(lambda self: _legalize_bir(orig_to_json()), nc)
    return nc


def _band(wt, kw, dbi, dbo):
    """[dbi*16, C, dbo*14] band matrix for one kw: B[(d_i,h_i), c, (d_o,h_o)]
    = wt[d_i-d_o, h_i-h_o, kw, c]."""
    out = np.zeros((dbi * HBI, C, dbo * HBO), np.float32)
    do = np.arange(dbo)
    ho = np.arange(HBO)
    po = (do[:, None] * HBO + ho[None, :]).ravel()
    for kd in range(KD):
        for kh in range(KH):
            pi = ((do[:, None] + kd) * HBI + ho[None, :] + kh).ravel()
            out[pi, :, po] = wt[kd, kh, kw, :]
    return out


def _host_prep(x: np.ndarray, w: np.ndarray):
    # x: (4, 16, 112, 112, 64) f32; w: (3, 3, 3, 1, 64) f32
    wt = w[:, :, :, 0, :].astype(np.float32)  # (kd, kh, kw, c)
    bm = np.stack([_band(wt, kw, DBI_M, DBO_M) for kw in range(KW)], axis=2)
    br = np.stack([_band(wt, kw, DBI_R, DBO_R) for kw in range(KW)], axis=2)
    bm = np.ascontiguousarray(bm.transpose(0, 1, 2, 3)).astype(BF16)  # [PM,C,KW,PMO]
    br = br.astype(BF16)

    xt = np.transpose(x, (0, 4, 1, 2, 3))  # (b, c, d, h, w)

    in_maps = []
    for k in range(N_CORES):
        b = k // 2
        h0 = (k % 2) * HH
        # padded input volume: d 18 (1 zero frame each side), h 58
        xp = np.zeros((C, D_FULL + 2, HH + 2, W), np.float32)
        hlo, hhi = h0 - 1, h0 + HH + 1
        chlo, chhi = max(hlo, 0), min(hhi, H)
        xp[:, 1 : D_FULL + 1, chlo - hlo : chlo - hlo + (chhi - chlo), :] = xt[
            b, :, :, chlo:chhi, :
        ]
        xm = np.empty((PM, C, 2, NHB, W), np.float32)
        xr = np.empty((PR, C, NHB, W), np.float32)
        for db in range(2):
            for hb in range(NHB):
                blk = xp[:, D0S[db] : D0S[db] + DBI_M, hb * HBO : hb * HBO + HBI, :]
                xm[:, :, db, hb, :] = blk.transpose(1, 2, 0, 3).reshape(PM, C, W)
        for hb in range(NHB):
            blk = xp[:, D0S[2] : D0S[2] + DBI_R, hb * HBO : hb * HBO + HBI, :]
            xr[:, :, hb, :] = blk.transpose(1, 2, 0, 3).reshape(PR, C, W)
        in_maps.append(
            {"xm": xm.astype(BF16), "xr": xr.astype(BF16), "bm": bm, "br": br}
        )
    return in_maps


def _assemble(results):
    y = np.empty((B_FULL, D_FULL, H, W, C), np.float32)
    for k in range(N_CORES):
        b = k // 2
        h0 = (k % 2) * HH
        ym = results[k]["ym"]  # [84, C, 2, 4, W]
        yr = results[k]["yr"]  # [56, C, 4, W]
        for db in range(2):
            for hb in range(NHB):
                blk = ym[:, :, db, hb, :].reshape(DBO_M, HBO, C, W)
                y[b, D0S[db] : D0S[db] + DBO_M, h0 + hb * HBO : h0 + (hb + 1) * HBO] = (
                    blk.transpose(0, 1, 3, 2)
                )
        for hb in range(NHB):
            blk = yr[:, :, hb, :].reshape(DBO_R, HBO, C, W)
            y[b, D0S[2] : D0S[2] + DBO_R, h0 + hb * HBO : h0 + (hb + 1) * HBO] = (
                blk.transpose(0, 1, 3, 2)
            )
    return y


def _run(x: np.ndarray, w: np.ndarray, trace: bool = False):
    from concourse.bass_utils import run_bass_kernel_spmd

    in_maps = _host_prep(np.asarray(x), np.asarray(w))
    nc = _build_nc()
    res = run_bass_kernel_spmd(nc, in_maps, list(range(N_CORES)), trace=trace)
    return _assemble(res.results), res.exec_time_ns


def kernel(x: np.ndarray, w: np.ndarray) -> np.ndarray:
    y, _ = _run(x, w, trace=False)
    return y


# revision 14
# speedup vs baseline: 1.7324x; 1.2656x over previous
"""Depthwise 3D conv (3x3x3, SAME, C=64) on 8 Trainium2 NeuronCores.

Strategy
--------
Data-parallel over (batch, h-half): core k handles b = k//2 and output
rows h in [56*(k%2), 56*(k%2)+56), all 16 d frames. Both the d-halo and
h-halo are materialized on host (zero-padded at volume edges), so every
core runs an identical program.

TensorE mapping: partitions carry a (d, h) block — input block (8, 16)
= 128 partitions, output block (6, 14) = 84 partitions — and the
stationary operand is a per-(channel, kw) banded matrix B[(d_i, h_i),
(d_o, h_o)] = w[kd = d_i - d_o, kh = h_i - h_o, kw, c], so ONE matmul
applies 9 of the 27 taps; the 3 kw taps are w-shifts on the moving
operand's access pattern, PSUM-accumulated. d = 16 tiles as output
blocks {6, 6, 4}; the ragged 4-block uses a (6, 16) = 96-partition
input block with its own (smaller) band matrices. h = 56 tiles as 4
blocks of 14, carried in the moving free dimension alongside w
(j = 4*112 = 448).

x is host-gathered into the block-partition layout (bf16), band
matrices built on host (bf16), outputs returned fp32.
"""

import json
import sys
import types

if "/opt/trn_rl_repo" not in sys.path:
    sys.path.insert(0, "/opt/trn_rl_repo")

import ml_dtypes
import numpy as np

KD = KH = KW = 3
C = 64
B_FULL, D_FULL, H, W = 4, 16, 112, 112
N_CORES = 8
HH = 56  # output h rows per core
NHB = 4  # h blocks of 14 per core
HBO = 14  # out h rows per block
HBI = 16  # in h rows per block
DBO_M, DBI_M = 6, 8  # main d block: out/in frames
DBO_R, DBI_R = 4, 6  # ragged d block
D0S = [0, 6, 12]  # out-frame starts of the 3 d blocks
PM = DBI_M * HBI  # 128 in-partitions (main)
PMO = DBO_M * HBO  # 84 out-partitions (main)
PR = DBI_R * HBI  # 96 in-partitions (ragged)
PRO = DBO_R * HBO  # 56 out-partitions (ragged)
CG = 8  # channels per input DMA chunk
OG = 4  # channels per output DMA chunk
F16 = np.float16

_KW_ORDER = [1, 0, 2]  # full-width tap first so PSUM start=True covers all cols


def _legalize_bir(raw: bytes) -> bytes:
    """walrus in this image caps sem waits at 1 per instruction; hoist extra
    waits onto preceding same-engine NoOps (sequencers run them in order)."""
    d = json.loads(raw)
    for fn in d["functions"]:
        for blk in fn["blocks"]:
            out = []
            for inst in blk["instructions"]:
                si = inst.get("sync_info")
                waits = (si or {}).get("on_wait") or []
                if len(waits) > 1:
                    for j, wt in enumerate(waits[:-1]):
                        out.append(
                            {
                                "debug": inst.get("debug", 0),
                                "engine": inst["engine"],
                                "ins": [],
                                "outs": [],
                                "name": f"{inst['name']}-w{j}",
                                "opcode": "NoOp",
                                "sync_info": {"on_wait": [wt], "on_update": []},
                            }
                        )
                    si["on_wait"] = [waits[-1]]
                out.append(inst)
            blk["instructions"] = out
    return json.dumps(d).encode()


def _w_ranges(kw):
    # out[w] += wt[kw] * x[w + kw - 1]
    if kw == 1:
        return 0, W, 0, W
    if kw == 0:
        return 0, W - 1, 1, W
    return 1, W, 0, W - 1


def _build_nc():
    import concourse.bass as bass
    import concourse.mybir as mybir
    import concourse.tile as tile

    nc = bass.Bass()
    xm_d = nc.declare_dram_parameter(
        "xm", [PM, C, 2, NHB, W], mybir.dt.float16, isOutput=False
    )
    xr_d = nc.declare_dram_parameter(
        "xr", [PR, C, NHB, W], mybir.dt.float16, isOutput=False
    )
    bm_d = nc.declare_dram_parameter(
        "bm", [PM, C, KW, PMO], mybir.dt.float16, isOutput=False
    )
    br_d = nc.declare_dram_parameter(
        "br", [PR, C, KW, PRO], mybir.dt.float16, isOutput=False
    )
    ym_d = nc.declare_dram_parameter(
        "ym", [PMO, C, 2, NHB, W], mybir.dt.float16, isOutput=True
    )
    yr_d = nc.declare_dram_parameter(
        "yr", [PRO, C, NHB, W], mybir.dt.float16, isOutput=True
    )

    with tile.TileContext(nc) as tc:
        with (
            tc.tile_pool(name="xin", bufs=3) as xin_pool,
            tc.tile_pool(name="bmat", bufs=3) as b_pool,
            tc.tile_pool(name="psum", bufs=5, space="PSUM") as psum_pool,
            tc.tile_pool(name="psumr", bufs=3, space="PSUM") as psumr_pool,
            tc.tile_pool(name="osb", bufs=3) as osb_pool,
        ):
            chunks = []
            c0 = 0
            for sz in [2, 2, 4] + [CG] * C:
                if c0 >= C:
                    break
                sz = min(sz, C - c0)
                chunks.append((c0, sz))
                c0 += sz
            for c0, csz in chunks:
                xm = xin_pool.tile([PM, CG, 2, NHB, W], mybir.dt.float16, tag="xm")
                xr = xin_pool.tile([PR, CG, NHB, W], mybir.dt.float16, tag="xr")
                bm = b_pool.tile([PM, CG, KW, PMO], mybir.dt.float16, tag="bm")
                br = b_pool.tile([PR, CG, KW, PRO], mybir.dt.float16, tag="br")
                nc.sync.dma_start(out=xm[:, :csz], in_=xm_d[:, c0 : c0 + csz])
                nc.sync.dma_start(out=xr[:, :csz], in_=xr_d[:, c0 : c0 + csz])
                nc.sync.dma_start(out=bm[:, :csz], in_=bm_d[:, c0 : c0 + csz])
                nc.sync.dma_start(out=br[:, :csz], in_=br_d[:, c0 : c0 + csz])
                for oi in range((csz + OG - 1) // OG):
                    og = min(OG, csz - oi * OG)
                    osm = osb_pool.tile([PMO, OG, 2, NHB, W], mybir.dt.float16, tag="osm")
                    osr = osb_pool.tile([PRO, OG, NHB, W], mybir.dt.float16, tag="osr")
                    for ci in range(og):
                        cc = oi * OG + ci
                        for db in range(2):
                            ps = psum_pool.tile([PMO, NHB, W], mybir.dt.float32, tag="psm")
                            for i, kw in enumerate(_KW_ORDER):
                                wi, wj, wo, wp = _w_ranges(kw)
                                nc.tensor.matmul(
                                    ps[:, :, wo:wp],
                                    bm[:, cc, kw, :],
                                    xm[:, cc, db, :, wi:wj],
                                    start=(i == 0),
                                    stop=(i == KW - 1),
                                    skip_group_check=(i != 0),
                                )
                            if db == 0:
                                nc.vector.tensor_copy(osm[:, ci, db], ps[:])
                            else:
                                nc.scalar.copy(out=osm[:, ci, db], in_=ps[:])
                        psr = psumr_pool.tile([PRO, NHB, W], mybir.dt.float32, tag="psr")
                        for i, kw in enumerate(_KW_ORDER):
                            wi, wj, wo, wp = _w_ranges(kw)
                            nc.tensor.matmul(
                                psr[:, :, wo:wp],
                                br[:, cc, kw, :],
                                xr[:, cc, :, wi:wj],
                                start=(i == 0),
                                stop=(i == KW - 1),
                                skip_group_check=(i != 0),
                            )
                        nc.vector.tensor_copy(osr[:, ci], psr[:])
                    yc0 = c0 + oi * OG
                    nc.scalar.dma_start(out=ym_d[:, yc0 : yc0 + og], in_=osm[:, :og])
                    nc.scalar.dma_start(out=yr_d[:, yc0 : yc0 + og], in_=osr[:, :og])

    orig_to_json = nc.to_json_bytes
    nc.to_json_bytes = types.MethodType(lambda self: _legalize_bir(orig_to_json()), nc)
    return nc


def _band(wt, kw, dbi, dbo):
    """[dbi*16, C, dbo*14] band matrix for one kw: B[(d_i,h_i), c, (d_o,h_o)]
    = wt[d_i-d_o, h_i-h_o, kw, c]."""
    out = np.zeros((dbi * HBI, C, dbo * HBO), np.float32)
    do = np.arange(dbo)
    ho = np.arange(HBO)
    po = (do[:, None] * HBO + ho[None, :]).ravel()
    for kd in range(KD):
        for kh in range(KH):
            pi = ((do[:, None] + kd) * HBI + ho[None, :] + kh).ravel()
            out[pi, :, po] = wt[kd, kh, kw, :]
    return out


def _host_prep(x: np.ndarray, w: np.ndarray):
    # x: (4, 16, 112, 112, 64) f32; w: (3, 3, 3, 1, 64) f32
    wt = w[:, :, :, 0, :].astype(np.float32)  # (kd, kh, kw, c)
    bm = np.stack([_band(wt, kw, DBI_M, DBO_M) for kw in range(KW)], axis=2)
    br = np.stack([_band(wt, kw, DBI_R, DBO_R) for kw in range(KW)], axis=2)
    bm = np.ascontiguousarray(bm.transpose(0, 1, 2, 3)).astype(F16)  # [PM,C,KW,PMO]
    br = br.astype(F16)

    xt = np.transpose(x, (0, 4, 1, 2, 3))  # (b, c, d, h, w)

    in_maps = []
    for k in range(N_CORES):
        b = k // 2
        h0 = (k % 2) * HH
        # padded input volume: d 18 (1 zero frame each side), h 58
        xp = np.zeros((C, D_FULL + 2, HH + 2, W), np.float32)
        hlo, hhi = h0 - 1, h0 + HH + 1
        chlo, chhi = max(hlo, 0), min(hhi, H)
        xp[:, 1 : D_FULL + 1, chlo - hlo : chlo - hlo + (chhi - chlo), :] = xt[
            b, :, :, chlo:chhi, :
        ]
        xm = np.empty((PM, C, 2, NHB, W), np.float32)
        xr = np.empty((PR, C, NHB, W), np.float32)
        for db in range(2):
            for hb in range(NHB):
                blk = xp[:, D0S[db] : D0S[db] + DBI_M, hb * HBO : hb * HBO + HBI, :]
                xm[:, :, db, hb, :] = blk.transpose(1, 2, 0, 3).reshape(PM, C, W)
        for hb in range(NHB):
            blk = xp[:, D0S[2] : D0S[2] + DBI_R, hb * HBO : hb * HBO + HBI, :]
            xr[:, :, hb, :] = blk.transpose(1, 2, 0, 3).reshape(PR, C, W)
        in_maps.append(
            {"xm": xm.astype(F16), "xr": xr.astype(F16), "bm": bm, "br": br}
        )
    return in_maps


def _assemble(results):
    y = np.empty((B_FULL, D_FULL, H, W, C), np.float32)
    for k in range(N_CORES):
        b = k // 2
        h0 = (k % 2) * HH
        ym = results[k]["ym"].astype(np.float32)  # [84, C, 2, 4, W]
        yr = results[k]["yr"].astype(np.float32)  # [56, C, 4, W]
        for db in range(2):
            for hb in range(NHB):
                blk = ym[:, :, db, hb, :].reshape(DBO_M, HBO, C, W)
                y[b, D0S[db] : D0S[db] + DBO_M, h0 + hb * HBO : h0 + (hb + 1) * HBO] = (
                    blk.transpose(0, 1, 3, 2)
                )
        for hb in range(NHB):
            blk = yr[:, :, hb, :].reshape(DBO_R, HBO, C, W)
            y[b, D0S[2] : D0S[2] + DBO_R, h0 + hb * HBO : h0 + (hb + 1) * HBO] = (
                blk.transpose(0, 1, 3, 2)
            )
    return y


def _run(x: np.ndarray, w: np.ndarray, trace: bool = False):
    from concourse.bass_utils import run_bass_kernel_spmd

    in_maps = _host_prep(np.asarray(x), np.asarray(w))
    nc = _build_nc()
    res = run_bass_kernel_spmd(nc, in_maps, list(range(N_CORES)), trace=trace)
    return _assemble(res.results), res.exec_time_ns


def kernel(x: np.ndarray, w: np.ndarray) -> np.ndarray:
    y, _ = _run(x, w, trace=False)
    return y
